# revision 28
# baseline (speedup 1.0000x reference)
"""Cross-attention (GQA, key-padding + shifted-causal mask) on 8 Trainium2 cores.

Sharding: core k handles kv head k for BOTH batches (4 query heads each under
GQA) -> 8 (b,h) attention instances per core, no collectives. This balances
work across cores because per-batch work depends on the ragged length.

Mask algebra: the reference adds -10000 for padded keys and replaces with
-10000 where s > t + len_b - Sk. With c_b = Sk - len_b the effective rule is
"key s visible to query t iff s <= t - c_b" (causality subsumes padding since
t - c_b <= len_b - 1 always). So per query chunk only the PREFIX of s-blocks
up to the causal diagonal participates; c_b is read from the runtime mask and
the program is compiled per (c_0, c_1) (cached). Queries t < c_b attend to
nothing; the reference gives them a uniform softmax -> mean(V), patched on
host.

v2 pipeline, per (b,h), score-transposed layout [s, t], TQ=256 query chunks,
s-blocks processed in DESCENDING s order, groups of GRP=6 per PSUM tile:
  ST = K^T Q            (bf16 matmuls; diagonal blocks trim dead t-prefix)
  P  = exp(scale*ST)    (one ScalarE call per group, fp16 out; the leading
                         dead t-prefix of the diagonal block is skipped)
  P *= diag_mask        (gpsimd affine_select on partially-masked blocks;
                         also zeroes the skipped/stale prefix)
  OT   += V'[s,d] P[s,t]   (fp16 matmuls, PSUM accum over s blocks)
  pacc6 += P_group      (VectorE fp16 adds into the FIRST group's p tile,
                         6 lanes; later folded to a 2-lane pacc2 segment in
                         a per-instance SBUF staging tile)
  OT psum -> osb (f16 cast) per chunk; osb + pacc staged per instance and
  DMA'd once (split for the last instance to cut tail latency).
The softmax denominator is finished on the HOST: den[t] = sum over the 128
partitions x 2 lanes of the pacc2 segment (fp32 numpy reduce), then
out = OT/den.  This removes all per-chunk denominator matmuls from the PE
and the small-tensor copies from VectorE.  Chunks with nb<=2 skip the fold:
their (masked) P tile is DMA'd directly into the pacc segment.
"""

import numpy as np

B, SQ, SK, H, HK, D = 2, 2048, 2048, 32, 8, 128
G = H // HK            # query heads per kv head
N_CORES = 8
TQ = 256               # t (query) tile width
TS = 128               # s (key) tile width
NTQ = SQ // TQ         # 8 t-chunks
GRP = 4                # s-blocks per exp group / ST psum tile
PW = 2 * TQ            # pacc2 segment width per chunk
SCALE = 1.0 / float(np.sqrt(D))

_compiled = {}


def _nb_table(c):
    """Number of s-blocks per t-chunk for shift c (prefix up to causal diag)."""
    nbmax = (SK - 1 - c) // TS + 1
    out = []
    for tc in range(NTQ):
        nb = (TQ * tc + TQ - 1 - c) // TS + 1
        out.append(min(max(nb, 0), nbmax))
    return out


def _build_program(c):
    """Build + schedule the SPMD Bass program, specialized on (c0, c1)."""
    from contextlib import ExitStack
    import concourse.bass as bass
    import concourse.tile as tile
    from concourse import bacc, mybir

    f32 = mybir.dt.float32
    bf16 = mybir.dt.bfloat16
    f16 = mybir.dt.float16

    nb_tabs = [_nb_table(ci) for ci in c]
    nbmaxs = [(SK - 1 - ci) // TS + 1 for ci in c]
    NBK = max(nbmaxs)
    t0s = [min(tc for tc in range(NTQ) if tab[tc] > 0) for tab in nb_tabs]

    nc = bacc.Bacc("TRN2", target_bir_lowering=False, debug=False)
    qT_ap = nc.dram_tensor("qT", [2 * G, D, SQ], bf16, kind="ExternalInput").ap()
    kT_ap = nc.dram_tensor("kT", [2, D, NBK * TS], bf16, kind="ExternalInput").ap()
    v_ap = nc.dram_tensor("v", [2, TS, NBK * D], f16, kind="ExternalInput").ap()
    ot_ap = nc.dram_tensor("ot", [2 * G, D, NTQ * TQ], f16,
                           kind="ExternalOutput").ap()
    pacc_ap = nc.dram_tensor("pacc", [2 * G, TS, NTQ * PW], f16,
                             kind="ExternalOutput").ap()

    with tile.TileContext(nc) as tc, ExitStack() as ctx:
        kv_pool = ctx.enter_context(tc.tile_pool(name="kv", bufs=2))
        q_pool = ctx.enter_context(tc.tile_pool(name="q", bufs=3))
        p_pool = ctx.enter_context(tc.tile_pool(name="p", bufs=6))
        osb_pool = ctx.enter_context(tc.tile_pool(name="osb", bufs=3))
        pacc_pool = ctx.enter_context(tc.tile_pool(name="pacc", bufs=3))
        st_psum = ctx.enter_context(tc.tile_pool(name="st", bufs=3, space="PSUM"))
        ot_psum = ctx.enter_context(tc.tile_pool(name="ot", bufs=2, space="PSUM"))

        # ---- input loads: the two tiles the first matmul needs go on the
        # sync queue (empty at start); everything else on the gpsimd queue so
        # output DMAs (sync) never park behind input loads.
        kT_sbs, v_sbs, qT_sbs = [], [], []
        for i in range(2):
            kT_sb = kv_pool.tile([D, NBK * TS], bf16, tag="kT")
            v_sb = kv_pool.tile([TS, NBK * D], f16, tag="v")
            kT_sbs.append(kT_sb)
            v_sbs.append(v_sb)
        for ih in range(2 * G):
            qT_sb = q_pool.tile([D, SQ], bf16, name=f"qT_sb{ih}")
            qT_sbs.append(qT_sb)

        # first chunk of instance 0 is tc=t0s[0]: needs kT block (nb-1..0) and
        # qT cols [t0*TQ, (t0+1)*TQ)
        first_tc = t0s[0]
        nc.sync.dma_start(kT_sbs[0][:, :TS], kT_ap[0][:, :TS])
        nc.sync.dma_start(
            qT_sbs[0][:, first_tc * TQ : (first_tc + 1) * TQ],
            qT_ap[0][:, first_tc * TQ : (first_tc + 1) * TQ],
        )
        nc.gpsimd.dma_start(kT_sbs[0][:, TS : nbmaxs[0] * TS],
                            kT_ap[0][:, TS : nbmaxs[0] * TS])

        def load_q(ih):
            i = ih // G
            lo = t0s[i] * TQ
            if ih == 0:
                # rest of head 0 on the sync queue (idle until outputs start)
                nc.sync.dma_start(
                    qT_sbs[0][:, (first_tc + 1) * TQ :],
                    qT_ap[0][:, (first_tc + 1) * TQ :],
                )
            else:
                nc.gpsimd.dma_start(qT_sbs[ih][:, lo:], qT_ap[ih][:, lo:])

        nc.gpsimd.dma_start(v_sbs[0][:, : 3 * D], v_ap[0][:, : 3 * D])
        load_q(0)
        nc.sync.dma_start(kT_sbs[1][:, : nbmaxs[1] * TS],
                          kT_ap[1][:, : nbmaxs[1] * TS])
        nc.gpsimd.dma_start(v_sbs[0][:, 3 * D : nbmaxs[0] * D],
                            v_ap[0][:, 3 * D : nbmaxs[0] * D])
        load_q(1)
        nc.gpsimd.dma_start(v_sbs[1][:, : nbmaxs[1] * D],
                            v_ap[1][:, : nbmaxs[1] * D])
        load_q(2)

        pending = None  # 1-deep SW pipeline keeps PE ahead of ACT

        def flush(pend):
            # PV matmuls for a finished group; on the chunk's last group also
            # emit the OT psum->sbuf cast-copy, the pacc fold (or direct P
            # DMA), and on the instance's last chunk the staged output DMAs.
            # Unmasked blocks go first so the PE never waits on the gpsimd
            # affine_select chain (all PV are full-width, so any block may
            # carry the start flag).
            order = ([u for u in range(pend["gn"]) if not pend["sel"][u]]
                     + [u for u in range(pend["gn"]) if pend["sel"][u]])
            for k, u in enumerate(order):
                sc = pend["blocks"][u]
                nc.tensor.matmul(
                    pend["ot_ps"][:, :TQ],
                    lhsT=pend["v_sb"][:, sc * D : (sc + 1) * D],
                    rhs=pend["p_sb"][:, u * TQ : (u + 1) * TQ],
                    start=(pend["first"] and k == 0),
                    stop=(pend["last"] and k == pend["gn"] - 1),
                )
            if pend["last"]:
                tcix, nb, ih = pend["tc"], pend["nb"], pend["ih"]
                osb, pacc_sb, pacc6 = pend["osb"], pend["pacc_sb"], pend["pacc6"]
                nc.vector.tensor_copy(
                    out=osb[:, tcix * TQ : (tcix + 1) * TQ],
                    in_=pend["ot_ps"][:, :TQ],
                )
                seg = pacc_sb[:, tcix * PW : (tcix + 1) * PW]
                if nb <= 2:
                    # masked P goes straight to DRAM; host sums nb*TQ cols
                    nc.sync.dma_start(
                        pacc_ap[ih][:, tcix * PW : tcix * PW + nb * TQ],
                        pacc6[:, : nb * TQ],
                    )
                else:
                    # fold the 4 lanes of pacc6 into 2 lanes (blocks beyond
                    # the first group were TT-accumulated into lanes mod 4;
                    # nb==3: the 4th lane was zeroed via memset)
                    nc.vector.tensor_tensor(
                        out=seg, in0=pacc6[:, :PW], in1=pacc6[:, PW : 2 * PW],
                        op=mybir.AluOpType.add,
                    )
                if ih >= 2 * G - 3:
                    # late instances: per-chunk DMAs so the output queue
                    # drains steadily and the final (smallest) chunk is the
                    # only tail work
                    nc.sync.dma_start(
                        ot_ap[ih][:, tcix * TQ : (tcix + 1) * TQ],
                        osb[:, tcix * TQ : (tcix + 1) * TQ])
                    if nb > 2:
                        q = nc.gpsimd if ih == 2 * G - 1 else nc.sync
                        q.dma_start(
                            pacc_ap[ih][:, tcix * PW : (tcix + 1) * PW], seg)
                elif pend["last_of_inst"]:
                    i = ih // G
                    lo_t = t0s[i]
                    fold0 = next(t for t in range(NTQ)
                                 if nb_tabs[i][t] > 2)  # first folded chunk
                    nc.sync.dma_start(ot_ap[ih][:, lo_t * TQ :],
                                      osb[:, lo_t * TQ :])
                    nc.sync.dma_start(pacc_ap[ih][:, fold0 * PW :],
                                      pacc_sb[:, fold0 * PW :])

        def chunk_order(live, ih):
            # interleave big/small chunks to balance the PE-heavy (many
            # blocks) and ACT-heavy (call overhead) phases; instance 0 goes
            # ascending (small first chunk -> fast start after DMA); the
            # last instance ends on the smallest chunk to cut tail latency.
            if ih == 0:
                return list(live)
            out = []
            if ih == 2 * G - 1:
                rest = live[1:]
                i0, j0 = 0, len(rest) - 1
                while i0 <= j0:
                    out.append(rest[j0])
                    if i0 < j0:
                        out.append(rest[i0])
                    i0 += 1
                    j0 -= 1
                return out + [live[0]]
            i0, j0 = 0, len(live) - 1
            while i0 <= j0:
                out.append(live[i0])
                if i0 < j0:
                    out.append(live[j0])
                i0 += 1
                j0 -= 1
            return out

        for i in range(2):  # batch
            cb = c[i]
            nb_tab = nb_tabs[i]
            kT_sb = kT_sbs[i]
            v_sb = v_sbs[i]

            for j in range(G):
                ih = i * G + j
                if ih + 2 < 2 * G:
                    load_q(ih + 2)
                qT_sb = qT_sbs[ih]
                osb = osb_pool.tile([D, NTQ * TQ], f16)
                pacc_sb = pacc_pool.tile([TS, NTQ * PW], f16)

                live = [t for t in range(NTQ) if nb_tab[t] > 0]
                order = chunk_order(live, ih)
                for oi, tcix in enumerate(order):
                    nb = nb_tab[tcix]
                    is_last_chunk = oi == len(order) - 1
                    ot_ps = ot_psum.tile([D, TQ], f32)
                    blocks_all = list(range(nb - 1, -1, -1))  # descending s
                    pacc6 = None
                    g0 = 0
                    while g0 < nb:
                        gn = min(GRP, nb - g0)
                        blocks = blocks_all[g0 : g0 + gn]
                        st_ps = st_psum.tile([TS, GRP * TQ], f32)
                        offs = []
                        for u, sc in enumerate(blocks):
                            bv = TS * sc + cb - TQ * tcix
                            off = max(0, min(bv, TQ))
                            offs.append((sc, bv, off))
                            if off < TQ:
                                nc.tensor.matmul(
                                    st_ps[:, u * TQ + off : (u + 1) * TQ],
                                    lhsT=kT_sb[:, sc * TS : (sc + 1) * TS],
                                    rhs=qT_sb[:, tcix * TQ + off :
                                              (tcix + 1) * TQ],
                                    start=True,
                                    stop=True,
                                )
                        p_sb = p_pool.tile([TS, GRP * TQ], f16)
                        if g0 == 0:
                            pacc6 = p_sb
                            if nb == 3:
                                nc.gpsimd.memset(p_sb[:, 3 * TQ : 4 * TQ], 0.0)
                        estart = offs[0][2] if g0 == 0 else 0
                        nc.scalar.activation(
                            p_sb[:, estart : gn * TQ],
                            st_ps[:, estart : gn * TQ],
                            mybir.ActivationFunctionType.Exp,
                            scale=SCALE,
                        )
                        sel = [bv > -(TS - 1) for (sc, bv, off) in offs]
                        for u, (sc, bv, off) in enumerate(offs):
                            if bv > -(TS - 1):  # partially masked block
                                nc.gpsimd.affine_select(
                                    out=p_sb[:, u * TQ : (u + 1) * TQ],
                                    in_=p_sb[:, u * TQ : (u + 1) * TQ],
                                    pattern=[[1, TQ]],
                                    compare_op=mybir.AluOpType.is_ge,
                                    fill=0.0,
                                    base=-bv,
                                    channel_multiplier=-1,
                                )
                        if pending is not None:
                            flush(pending)
                        if g0 > 0:
                            nc.vector.tensor_tensor(
                                out=pacc6[:, : gn * TQ],
                                in0=pacc6[:, : gn * TQ],
                                in1=p_sb[:, : gn * TQ],
                                op=mybir.AluOpType.add,
                            )
                        pending = {
                            "gn": gn, "nb": nb, "blocks": blocks, "sel": sel,
                            "p_sb": p_sb, "v_sb": v_sb, "ot_ps": ot_ps,
                            "pacc6": pacc6, "osb": osb, "pacc_sb": pacc_sb,
                            "ih": ih, "tc": tcix,
                            "first": g0 == 0, "last": g0 + gn >= nb,
                            "last_of_inst": (g0 + gn >= nb and is_last_chunk),
                        }
                        g0 += gn

        if pending is not None:
            flush(pending)

    nc.compile()
    return nc


def _get_program(c):
    key = tuple(int(x) for x in c)
    if key not in _compiled:
        _compiled[key] = _build_program(key)
    return _compiled[key]


def kernel(q, kv, key_padding_mask, _want_trace=False):
    import ml_dtypes

    bf16 = ml_dtypes.bfloat16
    q = np.asarray(q, dtype=np.float32)
    kv = np.asarray(kv, dtype=np.float32)
    mask = np.asarray(key_padding_mask).astype(bool)

    lengths = mask.sum(axis=1).astype(np.int64)
    # contiguous-prefix masks assumed (reference builds them that way)
    assert all(mask[b, : lengths[b]].all() and not mask[b, lengths[b]:].any()
               for b in range(B))
    c = tuple(int(SK - l) for l in lengths)
    nbmaxs = [(SK - 1 - ci) // TS + 1 for ci in c]
    NBK = max(nbmaxs)

    k_full = kv[:, :, 0]  # (B, SK, HK, D)
    v_full = kv[:, :, 1]

    k_bf = k_full.astype(bf16)
    v_16 = v_full.astype(np.float16)
    q_bf = q.astype(bf16)

    in_maps = []
    for core in range(N_CORES):
        hk = core
        qT = np.empty((2 * G, D, SQ), dtype=bf16)
        kT = np.zeros((2, D, NBK * TS), dtype=bf16)
        v_l = np.zeros((2, TS, NBK * D), dtype=np.float16)
        for i in range(B):
            nbk = nbmaxs[i]
            kT[i, :, : nbk * TS] = k_bf[i, : nbk * TS, hk, :].T
            # v chunked: v_l[i][p, sc*D + d] = v[i, sc*TS + p, hk, d]
            v_l[i, :, : nbk * D] = np.ascontiguousarray(
                v_16[i, : nbk * TS, hk, :].reshape(nbk, TS, D).transpose(1, 0, 2)
            ).reshape(TS, nbk * D)
            for j in range(G):
                qT[i * G + j] = q_bf[i, :, hk * G + j, :].T
        in_maps.append({
            "qT": np.ascontiguousarray(qT),
            "kT": kT,
            "v": v_l,
        })

    from concourse.bass_utils import run_bass_kernel_spmd

    nc = _get_program(c)
    res = run_bass_kernel_spmd(
        nc, in_maps, core_ids=list(range(N_CORES)),
        trace=_want_trace,
    )

    nb_tabs = [_nb_table(ci) for ci in c]
    out = np.empty((B, SQ, H, D), dtype=np.float32)
    for core in range(N_CORES):
        hk = core
        ot_core = res.results[core]["ot"]        # (2G, D, NTQ*TQ) f16
        pacc_core = res.results[core]["pacc"]    # (2G, TS, NTQ*PW) f16
        for i in range(B):
            nb_tab = nb_tabs[i]
            # den[t] from pacc segments: sum over partitions and live lanes
            pa = pacc_core[i * G : (i + 1) * G].astype(np.float32)
            pa = pa.reshape(G, TS, NTQ, 2, TQ)
            den = pa[:, :, :, 0].sum(axis=1)     # (G, NTQ, TQ)
            lane1 = pa[:, :, :, 1].sum(axis=1)
            for tc_i, nb_i in enumerate(nb_tab):
                if nb_i >= 2:
                    den[:, tc_i] += lane1[:, tc_i]
            den = den.reshape(G, SQ)
            ot = ot_core[i * G : (i + 1) * G].astype(np.float32)  # (G, D, SQ)
            with np.errstate(divide="ignore", invalid="ignore"):
                o = ot / den[:, None, :]          # (G, D, SQ)
            out[i, :, hk * G : (hk + 1) * G, :] = o.transpose(2, 0, 1)

    # rows that attend to nothing: reference softmax is uniform -> mean(V)
    for b in range(B):
        if c[b] > 0:
            vm = v_full[b].mean(axis=0)  # (HK, D)
            out[b, : c[b]] = np.repeat(vm, G, axis=0)[None]

    if _want_trace:
        return out, res
    return out


# revision 29
# speedup vs baseline: 1.0108x; 1.0108x over previous
"""Cross-attention (GQA, key-padding + shifted-causal mask) on 8 Trainium2 cores.

Sharding: core k handles kv head k for BOTH batches (4 query heads each under
GQA) -> 8 (b,h) attention instances per core, no collectives. This balances
work across cores because per-batch work depends on the ragged length.

Mask algebra: the reference adds -10000 for padded keys and replaces with
-10000 where s > t + len_b - Sk. With c_b = Sk - len_b the effective rule is
"key s visible to query t iff s <= t - c_b" (causality subsumes padding since
t - c_b <= len_b - 1 always). So per query chunk only the PREFIX of s-blocks
up to the causal diagonal participates; c_b is read from the runtime mask and
the program is compiled per (c_0, c_1) (cached). Queries t < c_b attend to
nothing; the reference gives them a uniform softmax -> mean(V), patched on
host.

v2 pipeline, per (b,h), score-transposed layout [s, t], TQ=256 query chunks,
s-blocks processed in DESCENDING s order, groups of GRP=6 per PSUM tile:
  ST = K^T Q            (bf16 matmuls; diagonal blocks trim dead t-prefix)
  P  = exp(scale*ST)    (one ScalarE call per group, fp16 out; the leading
                         dead t-prefix of the diagonal block is skipped)
  P *= diag_mask        (gpsimd affine_select on partially-masked blocks;
                         also zeroes the skipped/stale prefix)
  OT   += V'[s,d] P[s,t]   (fp16 matmuls, PSUM accum over s blocks)
  pacc6 += P_group      (VectorE fp16 adds into the FIRST group's p tile,
                         6 lanes; later folded to a 2-lane pacc2 segment in
                         a per-instance SBUF staging tile)
  OT psum -> osb (f16 cast) per chunk; osb + pacc staged per instance and
  DMA'd once (split for the last instance to cut tail latency).
The softmax denominator is finished on the HOST: den[t] = sum over the 128
partitions x 2 lanes of the pacc2 segment (fp32 numpy reduce), then
out = OT/den.  This removes all per-chunk denominator matmuls from the PE
and the small-tensor copies from VectorE.  Chunks with nb<=2 skip the fold:
their (masked) P tile is DMA'd directly into the pacc segment.
"""

import numpy as np

B, SQ, SK, H, HK, D = 2, 2048, 2048, 32, 8, 128
G = H // HK            # query heads per kv head
N_CORES = 8
TQ = 256               # t (query) tile width
TS = 128               # s (key) tile width
NTQ = SQ // TQ         # 8 t-chunks
GRP = 4                # s-blocks per exp group / ST psum tile
PW = 2 * TQ            # pacc2 segment width per chunk
SCALE = 1.0 / float(np.sqrt(D))

_compiled = {}


def _nb_table(c):
    """Number of s-blocks per t-chunk for shift c (prefix up to causal diag)."""
    nbmax = (SK - 1 - c) // TS + 1
    out = []
    for tc in range(NTQ):
        nb = (TQ * tc + TQ - 1 - c) // TS + 1
        out.append(min(max(nb, 0), nbmax))
    return out


def _build_program(c):
    """Build + schedule the SPMD Bass program, specialized on (c0, c1)."""
    from contextlib import ExitStack
    import concourse.bass as bass
    import concourse.tile as tile
    from concourse import bacc, mybir

    f32 = mybir.dt.float32
    bf16 = mybir.dt.bfloat16
    f16 = mybir.dt.float16

    nb_tabs = [_nb_table(ci) for ci in c]
    nbmaxs = [(SK - 1 - ci) // TS + 1 for ci in c]
    NBK = max(nbmaxs)
    t0s = [min(tc for tc in range(NTQ) if tab[tc] > 0) for tab in nb_tabs]

    nc = bacc.Bacc("TRN2", target_bir_lowering=False, debug=False)
    qT_ap = nc.dram_tensor("qT", [2 * G, D, SQ], bf16, kind="ExternalInput").ap()
    kT_ap = nc.dram_tensor("kT", [2, D, NBK * TS], bf16, kind="ExternalInput").ap()
    v_ap = nc.dram_tensor("v", [2, TS, NBK * D], f16, kind="ExternalInput").ap()
    ot_ap = nc.dram_tensor("ot", [2 * G, D, NTQ * TQ], f16,
                           kind="ExternalOutput").ap()
    pacc_ap = nc.dram_tensor("pacc", [2 * G, TS, NTQ * PW], f16,
                             kind="ExternalOutput").ap()

    with tile.TileContext(nc) as tc, ExitStack() as ctx:
        kv_pool = ctx.enter_context(tc.tile_pool(name="kv", bufs=2))
        q_pool = ctx.enter_context(tc.tile_pool(name="q", bufs=3))
        p_pool = ctx.enter_context(tc.tile_pool(name="p", bufs=6))
        osb_pool = ctx.enter_context(tc.tile_pool(name="osb", bufs=3))
        pacc_pool = ctx.enter_context(tc.tile_pool(name="pacc", bufs=3))
        st_psum = ctx.enter_context(tc.tile_pool(name="st", bufs=3, space="PSUM"))
        ot_psum = ctx.enter_context(tc.tile_pool(name="ot", bufs=2, space="PSUM"))

        # ---- input loads: the two tiles the first matmul needs go on the
        # sync queue (empty at start); everything else on the gpsimd queue so
        # output DMAs (sync) never park behind input loads.
        kT_sbs, v_sbs, qT_sbs = [], [], []
        for i in range(2):
            kT_sb = kv_pool.tile([D, NBK * TS], bf16, tag="kT")
            v_sb = kv_pool.tile([TS, NBK * D], f16, tag="v")
            kT_sbs.append(kT_sb)
            v_sbs.append(v_sb)
        for ih in range(2 * G):
            qT_sb = q_pool.tile([D, SQ], bf16, name=f"qT_sb{ih}")
            qT_sbs.append(qT_sb)

        # first chunk of instance 0 is tc=t0s[0]: needs kT block (nb-1..0) and
        # qT cols [t0*TQ, (t0+1)*TQ)
        first_tc = t0s[0]
        nc.sync.dma_start(kT_sbs[0][:, :TS], kT_ap[0][:, :TS])
        nc.sync.dma_start(
            qT_sbs[0][:, first_tc * TQ : (first_tc + 1) * TQ],
            qT_ap[0][:, first_tc * TQ : (first_tc + 1) * TQ],
        )
        nc.gpsimd.dma_start(kT_sbs[0][:, TS : nbmaxs[0] * TS],
                            kT_ap[0][:, TS : nbmaxs[0] * TS])

        def load_q(ih):
            i = ih // G
            lo = t0s[i] * TQ
            if ih == 0:
                # rest of head 0 (first chunk already on the sync queue)
                nc.gpsimd.dma_start(
                    qT_sbs[0][:, (first_tc + 1) * TQ :],
                    qT_ap[0][:, (first_tc + 1) * TQ :],
                )
            else:
                nc.gpsimd.dma_start(qT_sbs[ih][:, lo:], qT_ap[ih][:, lo:])

        nc.gpsimd.dma_start(v_sbs[0][:, : 3 * D], v_ap[0][:, : 3 * D])
        load_q(0)
        nc.gpsimd.dma_start(v_sbs[0][:, 3 * D : nbmaxs[0] * D],
                            v_ap[0][:, 3 * D : nbmaxs[0] * D])
        load_q(1)
        nc.gpsimd.dma_start(kT_sbs[1][:, : nbmaxs[1] * TS],
                            kT_ap[1][:, : nbmaxs[1] * TS])
        nc.gpsimd.dma_start(v_sbs[1][:, : nbmaxs[1] * D],
                            v_ap[1][:, : nbmaxs[1] * D])
        load_q(2)

        pending = None  # 1-deep SW pipeline keeps PE ahead of ACT

        def flush(pend):
            # PV matmuls for a finished group; on the chunk's last group also
            # emit the OT psum->sbuf cast-copy, the pacc fold (or direct P
            # DMA), and on the instance's last chunk the staged output DMAs.
            # Unmasked blocks go first so the PE never waits on the gpsimd
            # affine_select chain (all PV are full-width, so any block may
            # carry the start flag).
            order = ([u for u in range(pend["gn"]) if not pend["sel"][u]]
                     + [u for u in range(pend["gn"]) if pend["sel"][u]])
            for k, u in enumerate(order):
                sc = pend["blocks"][u]
                nc.tensor.matmul(
                    pend["ot_ps"][:, :TQ],
                    lhsT=pend["v_sb"][:, sc * D : (sc + 1) * D],
                    rhs=pend["p_sb"][:, u * TQ : (u + 1) * TQ],
                    start=(pend["first"] and k == 0),
                    stop=(pend["last"] and k == pend["gn"] - 1),
                )
            if pend["last"]:
                tcix, nb, ih = pend["tc"], pend["nb"], pend["ih"]
                osb, pacc_sb, pacc6 = pend["osb"], pend["pacc_sb"], pend["pacc6"]
                nc.vector.tensor_copy(
                    out=osb[:, tcix * TQ : (tcix + 1) * TQ],
                    in_=pend["ot_ps"][:, :TQ],
                )
                seg = pacc_sb[:, tcix * PW : (tcix + 1) * PW]
                if nb <= 2:
                    # masked P goes straight to DRAM; host sums nb*TQ cols
                    nc.sync.dma_start(
                        pacc_ap[ih][:, tcix * PW : tcix * PW + nb * TQ],
                        pacc6[:, : nb * TQ],
                    )
                else:
                    # fold the 4 lanes of pacc6 into 2 lanes (blocks beyond
                    # the first group were TT-accumulated into lanes mod 4;
                    # nb==3: the 4th lane was zeroed via memset)
                    nc.vector.tensor_tensor(
                        out=seg, in0=pacc6[:, :PW], in1=pacc6[:, PW : 2 * PW],
                        op=mybir.AluOpType.add,
                    )
                if ih == 2 * G - 1:
                    # last instance: per-chunk DMAs so the final (smallest)
                    # chunk is the only tail work
                    nc.sync.dma_start(
                        ot_ap[ih][:, tcix * TQ : (tcix + 1) * TQ],
                        osb[:, tcix * TQ : (tcix + 1) * TQ])
                    if nb > 2:
                        nc.sync.dma_start(
                            pacc_ap[ih][:, tcix * PW : (tcix + 1) * PW], seg)
                elif pend["last_of_inst"]:
                    i = ih // G
                    lo_t = t0s[i]
                    fold0 = next(t for t in range(NTQ)
                                 if nb_tabs[i][t] > 2)  # first folded chunk
                    nc.sync.dma_start(ot_ap[ih][:, lo_t * TQ :],
                                      osb[:, lo_t * TQ :])
                    nc.sync.dma_start(pacc_ap[ih][:, fold0 * PW :],
                                      pacc_sb[:, fold0 * PW :])

        def chunk_order(live, ih):
            # interleave big/small chunks to balance the PE-heavy (many
            # blocks) and ACT-heavy (call overhead) phases; instance 0 goes
            # ascending (small first chunk -> fast start after DMA); the
            # last instance ends on the smallest chunk to cut tail latency.
            if ih == 0:
                return list(live)
            out = []
            if ih == 2 * G - 1:
                rest = live[1:]
                i0, j0 = 0, len(rest) - 1
                while i0 <= j0:
                    out.append(rest[j0])
                    if i0 < j0:
                        out.append(rest[i0])
                    i0 += 1
                    j0 -= 1
                return out + [live[0]]
            i0, j0 = 0, len(live) - 1
            while i0 <= j0:
                out.append(live[i0])
                if i0 < j0:
                    out.append(live[j0])
                i0 += 1
                j0 -= 1
            return out

        for i in range(2):  # batch
            cb = c[i]
            nb_tab = nb_tabs[i]
            kT_sb = kT_sbs[i]
            v_sb = v_sbs[i]

            for j in range(G):
                ih = i * G + j
                if ih + 2 < 2 * G:
                    load_q(ih + 2)
                qT_sb = qT_sbs[ih]
                osb = osb_pool.tile([D, NTQ * TQ], f16)
                pacc_sb = pacc_pool.tile([TS, NTQ * PW], f16)

                live = [t for t in range(NTQ) if nb_tab[t] > 0]
                order = chunk_order(live, ih)
                for oi, tcix in enumerate(order):
                    nb = nb_tab[tcix]
                    is_last_chunk = oi == len(order) - 1
                    ot_ps = ot_psum.tile([D, TQ], f32)
                    blocks_all = list(range(nb - 1, -1, -1))  # descending s
                    pacc6 = None
                    g0 = 0
                    while g0 < nb:
                        gn = min(GRP, nb - g0)
                        blocks = blocks_all[g0 : g0 + gn]
                        st_ps = st_psum.tile([TS, GRP * TQ], f32)
                        offs = []
                        for u, sc in enumerate(blocks):
                            bv = TS * sc + cb - TQ * tcix
                            off = max(0, min(bv, TQ))
                            offs.append((sc, bv, off))
                            if off < TQ:
                                nc.tensor.matmul(
                                    st_ps[:, u * TQ + off : (u + 1) * TQ],
                                    lhsT=kT_sb[:, sc * TS : (sc + 1) * TS],
                                    rhs=qT_sb[:, tcix * TQ + off :
                                              (tcix + 1) * TQ],
                                    start=True,
                                    stop=True,
                                )
                        p_sb = p_pool.tile([TS, GRP * TQ], f16)
                        if g0 == 0:
                            pacc6 = p_sb
                            if nb == 3:
                                nc.gpsimd.memset(p_sb[:, 3 * TQ : 4 * TQ], 0.0)
                        estart = offs[0][2] if g0 == 0 else 0
                        nc.scalar.activation(
                            p_sb[:, estart : gn * TQ],
                            st_ps[:, estart : gn * TQ],
                            mybir.ActivationFunctionType.Exp,
                            scale=SCALE,
                        )
                        sel = [bv > -(TS - 1) for (sc, bv, off) in offs]
                        for u, (sc, bv, off) in enumerate(offs):
                            if bv > -(TS - 1):  # partially masked block
                                nc.gpsimd.affine_select(
                                    out=p_sb[:, u * TQ : (u + 1) * TQ],
                                    in_=p_sb[:, u * TQ : (u + 1) * TQ],
                                    pattern=[[1, TQ]],
                                    compare_op=mybir.AluOpType.is_ge,
                                    fill=0.0,
                                    base=-bv,
                                    channel_multiplier=-1,
                                )
                        if pending is not None:
                            flush(pending)
                        if g0 > 0:
                            nc.vector.tensor_tensor(
                                out=pacc6[:, : gn * TQ],
                                in0=pacc6[:, : gn * TQ],
                                in1=p_sb[:, : gn * TQ],
                                op=mybir.AluOpType.add,
                            )
                        pending = {
                            "gn": gn, "nb": nb, "blocks": blocks, "sel": sel,
                            "p_sb": p_sb, "v_sb": v_sb, "ot_ps": ot_ps,
                            "pacc6": pacc6, "osb": osb, "pacc_sb": pacc_sb,
                            "ih": ih, "tc": tcix,
                            "first": g0 == 0, "last": g0 + gn >= nb,
                            "last_of_inst": (g0 + gn >= nb and is_last_chunk),
                        }
                        g0 += gn

        if pending is not None:
            flush(pending)

    nc.compile()
    return nc


def _get_program(c):
    key = tuple(int(x) for x in c)
    if key not in _compiled:
        _compiled[key] = _build_program(key)
    return _compiled[key]


def kernel(q, kv, key_padding_mask, _want_trace=False):
    import ml_dtypes

    bf16 = ml_dtypes.bfloat16
    q = np.asarray(q, dtype=np.float32)
    kv = np.asarray(kv, dtype=np.float32)
    mask = np.asarray(key_padding_mask).astype(bool)

    lengths = mask.sum(axis=1).astype(np.int64)
    # contiguous-prefix masks assumed (reference builds them that way)
    assert all(mask[b, : lengths[b]].all() and not mask[b, lengths[b]:].any()
               for b in range(B))
    c = tuple(int(SK - l) for l in lengths)
    nbmaxs = [(SK - 1 - ci) // TS + 1 for ci in c]
    NBK = max(nbmaxs)

    k_full = kv[:, :, 0]  # (B, SK, HK, D)
    v_full = kv[:, :, 1]

    k_bf = k_full.astype(bf16)
    v_16 = v_full.astype(np.float16)
    q_bf = q.astype(bf16)

    in_maps = []
    for core in range(N_CORES):
        hk = core
        qT = np.empty((2 * G, D, SQ), dtype=bf16)
        kT = np.zeros((2, D, NBK * TS), dtype=bf16)
        v_l = np.zeros((2, TS, NBK * D), dtype=np.float16)
        for i in range(B):
            nbk = nbmaxs[i]
            kT[i, :, : nbk * TS] = k_bf[i, : nbk * TS, hk, :].T
            # v chunked: v_l[i][p, sc*D + d] = v[i, sc*TS + p, hk, d]
            v_l[i, :, : nbk * D] = np.ascontiguousarray(
                v_16[i, : nbk * TS, hk, :].reshape(nbk, TS, D).transpose(1, 0, 2)
            ).reshape(TS, nbk * D)
            for j in range(G):
                qT[i * G + j] = q_bf[i, :, hk * G + j, :].T
        in_maps.append({
            "qT": np.ascontiguousarray(qT),
            "kT": kT,
            "v": v_l,
        })

    from concourse.bass_utils import run_bass_kernel_spmd

    nc = _get_program(c)
    res = run_bass_kernel_spmd(
        nc, in_maps, core_ids=list(range(N_CORES)),
        trace=_want_trace,
    )

    nb_tabs = [_nb_table(ci) for ci in c]
    out = np.empty((B, SQ, H, D), dtype=np.float32)
    for core in range(N_CORES):
        hk = core
        ot_core = res.results[core]["ot"]        # (2G, D, NTQ*TQ) f16
        pacc_core = res.results[core]["pacc"]    # (2G, TS, NTQ*PW) f16
        for i in range(B):
            nb_tab = nb_tabs[i]
            # den[t] from pacc segments: sum over partitions and live lanes
            pa = pacc_core[i * G : (i + 1) * G].astype(np.float32)
            pa = pa.reshape(G, TS, NTQ, 2, TQ)
            den = pa[:, :, :, 0].sum(axis=1)     # (G, NTQ, TQ)
            lane1 = pa[:, :, :, 1].sum(axis=1)
            for tc_i, nb_i in enumerate(nb_tab):
                if nb_i >= 2:
                    den[:, tc_i] += lane1[:, tc_i]
            den = den.reshape(G, SQ)
            ot = ot_core[i * G : (i + 1) * G].astype(np.float32)  # (G, D, SQ)
            with np.errstate(divide="ignore", invalid="ignore"):
                o = ot / den[:, None, :]          # (G, D, SQ)
            out[i, :, hk * G : (hk + 1) * G, :] = o.transpose(2, 0, 1)

    # rows that attend to nothing: reference softmax is uniform -> mean(V)
    for b in range(B):
        if c[b] > 0:
            vm = v_full[b].mean(axis=0)  # (HK, D)
            out[b, : c[b]] = np.repeat(vm, G, axis=0)[None]

    if _want_trace:
        return out, res
    return out


# revision 30
# speedup vs baseline: 1.0449x; 1.0338x over previous
"""Cross-attention (GQA, key-padding + shifted-causal mask) on 8 Trainium2 cores.

Sharding: core k handles kv head k for BOTH batches (4 query heads each under
GQA) -> 8 (b,h) attention instances per core, no collectives. This balances
work across cores because per-batch work depends on the ragged length.

Mask algebra: the reference adds -10000 for padded keys and replaces with
-10000 where s > t + len_b - Sk. With c_b = Sk - len_b the effective rule is
"key s visible to query t iff s <= t - c_b" (causality subsumes padding since
t - c_b <= len_b - 1 always). So per query chunk only the PREFIX of s-blocks
up to the causal diagonal participates; c_b is read from the runtime mask and
the program is compiled per (c_0, c_1) (cached). Queries t < c_b attend to
nothing; the reference gives them a uniform softmax -> mean(V), patched on
host.

v2 pipeline, per (b,h), score-transposed layout [s, t], TQ=256 query chunks,
s-blocks processed in DESCENDING s order, groups of GRP=6 per PSUM tile:
  ST = K^T Q            (bf16 matmuls; diagonal blocks trim dead t-prefix)
  P  = exp(scale*ST)    (one ScalarE call per group, fp16 out; the leading
                         dead t-prefix of the diagonal block is skipped)
  P *= diag_mask        (gpsimd affine_select on partially-masked blocks;
                         also zeroes the skipped/stale prefix)
  OT   += V'[s,d] P[s,t]   (fp16 matmuls, PSUM accum over s blocks)
  pacc6 += P_group      (VectorE fp16 adds into the FIRST group's p tile,
                         6 lanes; later folded to a 2-lane pacc2 segment in
                         a per-instance SBUF staging tile)
  OT psum -> osb (f16 cast) per chunk; osb + pacc staged per instance and
  DMA'd once (split for the last instance to cut tail latency).
The softmax denominator is finished on the HOST: den[t] = sum over the 128
partitions x 2 lanes of the pacc2 segment (fp32 numpy reduce), then
out = OT/den.  This removes all per-chunk denominator matmuls from the PE
and the small-tensor copies from VectorE.  Chunks with nb<=2 skip the fold:
their (masked) P tile is DMA'd directly into the pacc segment.
"""

import numpy as np

B, SQ, SK, H, HK, D = 2, 2048, 2048, 32, 8, 128
G = H // HK            # query heads per kv head
N_CORES = 8
TQ = 256               # t (query) tile width
TS = 128               # s (key) tile width
NTQ = SQ // TQ         # 8 t-chunks
GRP = 4                # s-blocks per exp group / ST psum tile
PW = 2 * TQ            # pacc2 segment width per chunk
SCALE = 1.0 / float(np.sqrt(D))

_compiled = {}


def _nb_table(c):
    """Number of s-blocks per t-chunk for shift c (prefix up to causal diag)."""
    nbmax = (SK - 1 - c) // TS + 1
    out = []
    for tc in range(NTQ):
        nb = (TQ * tc + TQ - 1 - c) // TS + 1
        out.append(min(max(nb, 0), nbmax))
    return out


def _build_program(c):
    """Build + schedule the SPMD Bass program, specialized on (c0, c1)."""
    from contextlib import ExitStack
    import concourse.bass as bass
    import concourse.tile as tile
    from concourse import bacc, mybir

    f32 = mybir.dt.float32
    bf16 = mybir.dt.bfloat16
    f16 = mybir.dt.float16

    nb_tabs = [_nb_table(ci) for ci in c]
    nbmaxs = [(SK - 1 - ci) // TS + 1 for ci in c]
    NBK = max(nbmaxs)
    t0s = [min(tc for tc in range(NTQ) if tab[tc] > 0) for tab in nb_tabs]

    nc = bacc.Bacc("TRN2", target_bir_lowering=False, debug=False)
    qT_ap = nc.dram_tensor("qT", [2 * G, D, SQ], bf16, kind="ExternalInput").ap()
    kT_ap = nc.dram_tensor("kT", [2, D, NBK * TS], bf16, kind="ExternalInput").ap()
    v_ap = nc.dram_tensor("v", [2, TS, NBK * D], f16, kind="ExternalInput").ap()
    ot_ap = nc.dram_tensor("ot", [2 * G, D, NTQ * TQ], f16,
                           kind="ExternalOutput").ap()
    pacc_ap = nc.dram_tensor("pacc", [2 * G, TS, NTQ * PW], f16,
                             kind="ExternalOutput").ap()

    with tile.TileContext(nc) as tc, ExitStack() as ctx:
        kv_pool = ctx.enter_context(tc.tile_pool(name="kv", bufs=2))
        q_pool = ctx.enter_context(tc.tile_pool(name="q", bufs=3))
        p_pool = ctx.enter_context(tc.tile_pool(name="p", bufs=6))
        osb_pool = ctx.enter_context(tc.tile_pool(name="osb", bufs=3))
        pacc_pool = ctx.enter_context(tc.tile_pool(name="pacc", bufs=3))
        st_psum = ctx.enter_context(tc.tile_pool(name="st", bufs=3, space="PSUM"))
        ot_psum = ctx.enter_context(tc.tile_pool(name="ot", bufs=2, space="PSUM"))

        # ---- input loads: the two tiles the first matmul needs go on the
        # sync queue (empty at start); everything else on the gpsimd queue so
        # output DMAs (sync) never park behind input loads.
        kT_sbs, v_sbs, qT_sbs = [], [], []
        for i in range(2):
            kT_sb = kv_pool.tile([D, NBK * TS], bf16, tag="kT")
            v_sb = kv_pool.tile([TS, NBK * D], f16, tag="v")
            kT_sbs.append(kT_sb)
            v_sbs.append(v_sb)
        for ih in range(2 * G):
            qT_sb = q_pool.tile([D, SQ], bf16, name=f"qT_sb{ih}")
            qT_sbs.append(qT_sb)

        # first chunk of instance 0 is tc=t0s[0]: needs kT block (nb-1..0) and
        # qT cols [t0*TQ, (t0+1)*TQ)
        first_tc = t0s[0]
        # whole kT0 in one load: contiguous ~3.3KB dram rows hit DMA line
        # rate, while a narrow [D, TS] slice (256B rows) crawls
        nc.sync.dma_start(kT_sbs[0][:, : nbmaxs[0] * TS],
                          kT_ap[0][:, : nbmaxs[0] * TS])
        nc.sync.dma_start(
            qT_sbs[0][:, first_tc * TQ : (first_tc + 4) * TQ],
            qT_ap[0][:, first_tc * TQ : (first_tc + 4) * TQ],
        )

        def load_q(ih):
            i = ih // G
            lo = t0s[i] * TQ
            if ih == 0:
                # rest of head 0 (first chunks already on the sync queue)
                nc.gpsimd.dma_start(
                    qT_sbs[0][:, (first_tc + 4) * TQ :],
                    qT_ap[0][:, (first_tc + 4) * TQ :],
                )
            else:
                nc.gpsimd.dma_start(qT_sbs[ih][:, lo:], qT_ap[ih][:, lo:])

        nc.gpsimd.dma_start(v_sbs[0][:, : 3 * D], v_ap[0][:, : 3 * D])
        load_q(0)
        nc.gpsimd.dma_start(v_sbs[0][:, 3 * D : nbmaxs[0] * D],
                            v_ap[0][:, 3 * D : nbmaxs[0] * D])
        load_q(1)
        nc.gpsimd.dma_start(kT_sbs[1][:, : nbmaxs[1] * TS],
                            kT_ap[1][:, : nbmaxs[1] * TS])
        nc.gpsimd.dma_start(v_sbs[1][:, : nbmaxs[1] * D],
                            v_ap[1][:, : nbmaxs[1] * D])
        load_q(2)

        pending = None  # 1-deep SW pipeline keeps PE ahead of ACT

        def flush(pend):
            # PV matmuls for a finished group; on the chunk's last group also
            # emit the OT psum->sbuf cast-copy, the pacc fold (or direct P
            # DMA), and on the instance's last chunk the staged output DMAs.
            # Unmasked blocks go first so the PE never waits on the gpsimd
            # affine_select chain (all PV are full-width, so any block may
            # carry the start flag).
            order = ([u for u in range(pend["gn"]) if not pend["sel"][u]]
                     + [u for u in range(pend["gn"]) if pend["sel"][u]])
            for k, u in enumerate(order):
                sc = pend["blocks"][u]
                nc.tensor.matmul(
                    pend["ot_ps"][:, :TQ],
                    lhsT=pend["v_sb"][:, sc * D : (sc + 1) * D],
                    rhs=pend["p_sb"][:, u * TQ : (u + 1) * TQ],
                    start=(pend["first"] and k == 0),
                    stop=(pend["last"] and k == pend["gn"] - 1),
                )
            if pend["last"]:
                tcix, nb, ih = pend["tc"], pend["nb"], pend["ih"]
                osb, pacc_sb, pacc6 = pend["osb"], pend["pacc_sb"], pend["pacc6"]
                nc.vector.tensor_copy(
                    out=osb[:, tcix * TQ : (tcix + 1) * TQ],
                    in_=pend["ot_ps"][:, :TQ],
                )
                seg = pacc_sb[:, tcix * PW : (tcix + 1) * PW]
                if nb <= 2:
                    # masked P goes straight to DRAM; host sums nb*TQ cols
                    nc.sync.dma_start(
                        pacc_ap[ih][:, tcix * PW : tcix * PW + nb * TQ],
                        pacc6[:, : nb * TQ],
                    )
                else:
                    # fold the 4 lanes of pacc6 into 2 lanes (blocks beyond
                    # the first group were TT-accumulated into lanes mod 4;
                    # nb==3: the 4th lane was zeroed via memset)
                    nc.vector.tensor_tensor(
                        out=seg, in0=pacc6[:, :PW], in1=pacc6[:, PW : 2 * PW],
                        op=mybir.AluOpType.add,
                    )
                if ih == 2 * G - 1:
                    # last instance: per-chunk DMAs so the final (smallest)
                    # chunk is the only tail work
                    nc.sync.dma_start(
                        ot_ap[ih][:, tcix * TQ : (tcix + 1) * TQ],
                        osb[:, tcix * TQ : (tcix + 1) * TQ])
                    if nb > 2:
                        nc.sync.dma_start(
                            pacc_ap[ih][:, tcix * PW : (tcix + 1) * PW], seg)
                elif pend["last_of_inst"]:
                    i = ih // G
                    lo_t = t0s[i]
                    fold0 = next(t for t in range(NTQ)
                                 if nb_tabs[i][t] > 2)  # first folded chunk
                    nc.sync.dma_start(ot_ap[ih][:, lo_t * TQ :],
                                      osb[:, lo_t * TQ :])
                    nc.sync.dma_start(pacc_ap[ih][:, fold0 * PW :],
                                      pacc_sb[:, fold0 * PW :])

        def chunk_order(live, ih):
            # interleave big/small chunks to balance the PE-heavy (many
            # blocks) and ACT-heavy (call overhead) phases; instance 0 goes
            # ascending (small first chunk -> fast start after DMA); the
            # last instance ends on the smallest chunk to cut tail latency.
            if ih == 0:
                return list(live)
            out = []
            if ih == 2 * G - 1:
                rest = live[1:]
                i0, j0 = 0, len(rest) - 1
                while i0 <= j0:
                    out.append(rest[j0])
                    if i0 < j0:
                        out.append(rest[i0])
                    i0 += 1
                    j0 -= 1
                return out + [live[0]]
            i0, j0 = 0, len(live) - 1
            while i0 <= j0:
                out.append(live[i0])
                if i0 < j0:
                    out.append(live[j0])
                i0 += 1
                j0 -= 1
            return out

        for i in range(2):  # batch
            cb = c[i]
            nb_tab = nb_tabs[i]
            kT_sb = kT_sbs[i]
            v_sb = v_sbs[i]

            for j in range(G):
                ih = i * G + j
                if ih + 2 < 2 * G:
                    load_q(ih + 2)
                qT_sb = qT_sbs[ih]
                osb = osb_pool.tile([D, NTQ * TQ], f16)
                pacc_sb = pacc_pool.tile([TS, NTQ * PW], f16)

                live = [t for t in range(NTQ) if nb_tab[t] > 0]
                order = chunk_order(live, ih)
                for oi, tcix in enumerate(order):
                    nb = nb_tab[tcix]
                    is_last_chunk = oi == len(order) - 1
                    ot_ps = ot_psum.tile([D, TQ], f32)
                    blocks_all = list(range(nb - 1, -1, -1))  # descending s
                    pacc6 = None
                    g0 = 0
                    while g0 < nb:
                        gn = min(GRP, nb - g0)
                        blocks = blocks_all[g0 : g0 + gn]
                        st_ps = st_psum.tile([TS, GRP * TQ], f32)
                        offs = []
                        for u, sc in enumerate(blocks):
                            bv = TS * sc + cb - TQ * tcix
                            off = max(0, min(bv, TQ))
                            offs.append((sc, bv, off))
                            if off < TQ:
                                nc.tensor.matmul(
                                    st_ps[:, u * TQ + off : (u + 1) * TQ],
                                    lhsT=kT_sb[:, sc * TS : (sc + 1) * TS],
                                    rhs=qT_sb[:, tcix * TQ + off :
                                              (tcix + 1) * TQ],
                                    start=True,
                                    stop=True,
                                )
                        p_sb = p_pool.tile([TS, GRP * TQ], f16)
                        if g0 == 0:
                            pacc6 = p_sb
                            if nb == 3:
                                nc.gpsimd.memset(p_sb[:, 3 * TQ : 4 * TQ], 0.0)
                        estart = offs[0][2] if g0 == 0 else 0
                        nc.scalar.activation(
                            p_sb[:, estart : gn * TQ],
                            st_ps[:, estart : gn * TQ],
                            mybir.ActivationFunctionType.Exp,
                            scale=SCALE,
                        )
                        sel = [bv > -(TS - 1) for (sc, bv, off) in offs]
                        for u, (sc, bv, off) in enumerate(offs):
                            if bv > -(TS - 1):  # partially masked block
                                nc.gpsimd.affine_select(
                                    out=p_sb[:, u * TQ : (u + 1) * TQ],
                                    in_=p_sb[:, u * TQ : (u + 1) * TQ],
                                    pattern=[[1, TQ]],
                                    compare_op=mybir.AluOpType.is_ge,
                                    fill=0.0,
                                    base=-bv,
                                    channel_multiplier=-1,
                                )
                        if pending is not None:
                            flush(pending)
                        if g0 > 0:
                            nc.vector.tensor_tensor(
                                out=pacc6[:, : gn * TQ],
                                in0=pacc6[:, : gn * TQ],
                                in1=p_sb[:, : gn * TQ],
                                op=mybir.AluOpType.add,
                            )
                        pending = {
                            "gn": gn, "nb": nb, "blocks": blocks, "sel": sel,
                            "p_sb": p_sb, "v_sb": v_sb, "ot_ps": ot_ps,
                            "pacc6": pacc6, "osb": osb, "pacc_sb": pacc_sb,
                            "ih": ih, "tc": tcix,
                            "first": g0 == 0, "last": g0 + gn >= nb,
                            "last_of_inst": (g0 + gn >= nb and is_last_chunk),
                        }
                        g0 += gn

        if pending is not None:
            flush(pending)

    nc.compile()
    return nc


def _get_program(c):
    key = tuple(int(x) for x in c)
    if key not in _compiled:
        _compiled[key] = _build_program(key)
    return _compiled[key]


def kernel(q, kv, key_padding_mask, _want_trace=False):
    import ml_dtypes

    bf16 = ml_dtypes.bfloat16
    q = np.asarray(q, dtype=np.float32)
    kv = np.asarray(kv, dtype=np.float32)
    mask = np.asarray(key_padding_mask).astype(bool)

    lengths = mask.sum(axis=1).astype(np.int64)
    # contiguous-prefix masks assumed (reference builds them that way)
    assert all(mask[b, : lengths[b]].all() and not mask[b, lengths[b]:].any()
               for b in range(B))
    c = tuple(int(SK - l) for l in lengths)
    nbmaxs = [(SK - 1 - ci) // TS + 1 for ci in c]
    NBK = max(nbmaxs)

    k_full = kv[:, :, 0]  # (B, SK, HK, D)
    v_full = kv[:, :, 1]

    k_bf = k_full.astype(bf16)
    v_16 = v_full.astype(np.float16)
    q_bf = q.astype(bf16)

    in_maps = []
    for core in range(N_CORES):
        hk = core
        qT = np.empty((2 * G, D, SQ), dtype=bf16)
        kT = np.zeros((2, D, NBK * TS), dtype=bf16)
        v_l = np.zeros((2, TS, NBK * D), dtype=np.float16)
        for i in range(B):
            nbk = nbmaxs[i]
            kT[i, :, : nbk * TS] = k_bf[i, : nbk * TS, hk, :].T
            # v chunked: v_l[i][p, sc*D + d] = v[i, sc*TS + p, hk, d]
            v_l[i, :, : nbk * D] = np.ascontiguousarray(
                v_16[i, : nbk * TS, hk, :].reshape(nbk, TS, D).transpose(1, 0, 2)
            ).reshape(TS, nbk * D)
            for j in range(G):
                qT[i * G + j] = q_bf[i, :, hk * G + j, :].T
        in_maps.append({
            "qT": np.ascontiguousarray(qT),
            "kT": kT,
            "v": v_l,
        })

    from concourse.bass_utils import run_bass_kernel_spmd

    nc = _get_program(c)
    res = run_bass_kernel_spmd(
        nc, in_maps, core_ids=list(range(N_CORES)),
        trace=_want_trace,
    )

    nb_tabs = [_nb_table(ci) for ci in c]
    out = np.empty((B, SQ, H, D), dtype=np.float32)
    for core in range(N_CORES):
        hk = core
        ot_core = res.results[core]["ot"]        # (2G, D, NTQ*TQ) f16
        pacc_core = res.results[core]["pacc"]    # (2G, TS, NTQ*PW) f16
        for i in range(B):
            nb_tab = nb_tabs[i]
            # den[t] from pacc segments: sum over partitions and live lanes
            pa = pacc_core[i * G : (i + 1) * G].astype(np.float32)
            pa = pa.reshape(G, TS, NTQ, 2, TQ)
            den = pa[:, :, :, 0].sum(axis=1)     # (G, NTQ, TQ)
            lane1 = pa[:, :, :, 1].sum(axis=1)
            for tc_i, nb_i in enumerate(nb_tab):
                if nb_i >= 2:
                    den[:, tc_i] += lane1[:, tc_i]
            den = den.reshape(G, SQ)
            ot = ot_core[i * G : (i + 1) * G].astype(np.float32)  # (G, D, SQ)
            with np.errstate(divide="ignore", invalid="ignore"):
                o = ot / den[:, None, :]          # (G, D, SQ)
            out[i, :, hk * G : (hk + 1) * G, :] = o.transpose(2, 0, 1)

    # rows that attend to nothing: reference softmax is uniform -> mean(V)
    for b in range(B):
        if c[b] > 0:
            vm = v_full[b].mean(axis=0)  # (HK, D)
            out[b, : c[b]] = np.repeat(vm, G, axis=0)[None]

    if _want_trace:
        return out, res
    return out


# revision 32
# speedup vs baseline: 1.0509x; 1.0057x over previous
"""Cross-attention (GQA, key-padding + shifted-causal mask) on 8 Trainium2 cores.

Sharding: core k handles kv head k for BOTH batches (4 query heads each under
GQA) -> 8 (b,h) attention instances per core, no collectives. This balances
work across cores because per-batch work depends on the ragged length.

Mask algebra: the reference adds -10000 for padded keys and replaces with
-10000 where s > t + len_b - Sk. With c_b = Sk - len_b the effective rule is
"key s visible to query t iff s <= t - c_b" (causality subsumes padding since
t - c_b <= len_b - 1 always). So per query chunk only the PREFIX of s-blocks
up to the causal diagonal participates; c_b is read from the runtime mask and
the program is compiled per (c_0, c_1) (cached). Queries t < c_b attend to
nothing; the reference gives them a uniform softmax -> mean(V), patched on
host.

v2 pipeline, per (b,h), score-transposed layout [s, t], TQ=256 query chunks,
s-blocks processed in DESCENDING s order, groups of GRP=6 per PSUM tile:
  ST = K^T Q            (bf16 matmuls; diagonal blocks trim dead t-prefix)
  P  = exp(scale*ST)    (one ScalarE call per group, fp16 out; the leading
                         dead t-prefix of the diagonal block is skipped)
  P *= diag_mask        (gpsimd affine_select on partially-masked blocks;
                         also zeroes the skipped/stale prefix)
  OT   += V'[s,d] P[s,t]   (fp16 matmuls, PSUM accum over s blocks)
  pacc6 += P_group      (VectorE fp16 adds into the FIRST group's p tile,
                         6 lanes; later folded to a 2-lane pacc2 segment in
                         a per-instance SBUF staging tile)
  OT psum -> osb (f16 cast) per chunk; osb + pacc staged per instance and
  DMA'd once (split for the last instance to cut tail latency).
The softmax denominator is finished on the HOST: den[t] = sum over the 128
partitions x 2 lanes of the pacc2 segment (fp32 numpy reduce), then
out = OT/den.  This removes all per-chunk denominator matmuls from the PE
and the small-tensor copies from VectorE.  Chunks with nb<=2 skip the fold:
their (masked) P tile is DMA'd directly into the pacc segment.
"""

import numpy as np

B, SQ, SK, H, HK, D = 2, 2048, 2048, 32, 8, 128
G = H // HK            # query heads per kv head
N_CORES = 8
TQ = 256               # t (query) tile width
TS = 128               # s (key) tile width
NTQ = SQ // TQ         # 8 t-chunks
GRP = 4                # s-blocks per exp group / ST psum tile
PW = 2 * TQ            # pacc2 segment width per chunk
SCALE = 1.0 / float(np.sqrt(D))

_compiled = {}


def _nb_table(c):
    """Number of s-blocks per t-chunk for shift c (prefix up to causal diag)."""
    nbmax = (SK - 1 - c) // TS + 1
    out = []
    for tc in range(NTQ):
        nb = (TQ * tc + TQ - 1 - c) // TS + 1
        out.append(min(max(nb, 0), nbmax))
    return out


def _build_program(c):
    """Build + schedule the SPMD Bass program, specialized on (c0, c1)."""
    from contextlib import ExitStack
    import concourse.bass as bass
    import concourse.tile as tile
    from concourse import bacc, mybir

    f32 = mybir.dt.float32
    bf16 = mybir.dt.bfloat16
    f16 = mybir.dt.float16

    nb_tabs = [_nb_table(ci) for ci in c]
    nbmaxs = [(SK - 1 - ci) // TS + 1 for ci in c]
    NBK = max(nbmaxs)
    t0s = [min(tc for tc in range(NTQ) if tab[tc] > 0) for tab in nb_tabs]

    nc = bacc.Bacc("TRN2", target_bir_lowering=False, debug=False)
    qT_ap = nc.dram_tensor("qT", [2 * G, D, SQ], bf16, kind="ExternalInput").ap()
    kT_ap = nc.dram_tensor("kT", [2, D, NBK * TS], bf16, kind="ExternalInput").ap()
    v_ap = nc.dram_tensor("v", [2, TS, NBK * D], f16, kind="ExternalInput").ap()
    ot_ap = nc.dram_tensor("ot", [2 * G, D, NTQ * TQ], f16,
                           kind="ExternalOutput").ap()
    pacc_ap = nc.dram_tensor("pacc", [2 * G, TS, NTQ * PW], f16,
                             kind="ExternalOutput").ap()

    with tile.TileContext(nc) as tc, ExitStack() as ctx:
        kv_pool = ctx.enter_context(tc.tile_pool(name="kv", bufs=2))
        q_pool = ctx.enter_context(tc.tile_pool(name="q", bufs=3))
        p_pool = ctx.enter_context(tc.tile_pool(name="p", bufs=7))
        osb_pool = ctx.enter_context(tc.tile_pool(name="osb", bufs=3))
        pacc_pool = ctx.enter_context(tc.tile_pool(name="pacc", bufs=3))
        st_psum = ctx.enter_context(tc.tile_pool(name="st", bufs=3, space="PSUM"))
        ot_psum = ctx.enter_context(tc.tile_pool(name="ot", bufs=2, space="PSUM"))

        # ---- input loads: the two tiles the first matmul needs go on the
        # sync queue (empty at start); everything else on the gpsimd queue so
        # output DMAs (sync) never park behind input loads.
        kT_sbs, v_sbs, qT_sbs = [], [], []
        for i in range(2):
            kT_sb = kv_pool.tile([D, NBK * TS], bf16, tag="kT")
            v_sb = kv_pool.tile([TS, NBK * D], f16, tag="v")
            kT_sbs.append(kT_sb)
            v_sbs.append(v_sb)
        for ih in range(2 * G):
            qT_sb = q_pool.tile([D, SQ], bf16, name=f"qT_sb{ih}")
            qT_sbs.append(qT_sb)

        # first chunk of instance 0 is tc=t0s[0]: needs kT block (nb-1..0) and
        # qT cols [t0*TQ, (t0+1)*TQ)
        first_tc = t0s[0]
        # whole kT0 in one load: contiguous ~3.3KB dram rows hit DMA line
        # rate, while a narrow [D, TS] slice (256B rows) crawls
        nc.sync.dma_start(kT_sbs[0][:, : nbmaxs[0] * TS],
                          kT_ap[0][:, : nbmaxs[0] * TS])
        nc.sync.dma_start(
            qT_sbs[0][:, first_tc * TQ : (first_tc + 4) * TQ],
            qT_ap[0][:, first_tc * TQ : (first_tc + 4) * TQ],
        )

        def load_q(ih):
            i = ih // G
            lo = t0s[i] * TQ
            if ih == 0:
                # rest of head 0 (first chunks already on the sync queue)
                nc.gpsimd.dma_start(
                    qT_sbs[0][:, (first_tc + 4) * TQ :],
                    qT_ap[0][:, (first_tc + 4) * TQ :],
                )
            else:
                nc.gpsimd.dma_start(qT_sbs[ih][:, lo:], qT_ap[ih][:, lo:])

        nc.gpsimd.dma_start(v_sbs[0][:, : 3 * D], v_ap[0][:, : 3 * D])
        load_q(0)
        nc.gpsimd.dma_start(v_sbs[0][:, 3 * D : nbmaxs[0] * D],
                            v_ap[0][:, 3 * D : nbmaxs[0] * D])
        load_q(1)
        nc.gpsimd.dma_start(kT_sbs[1][:, : nbmaxs[1] * TS],
                            kT_ap[1][:, : nbmaxs[1] * TS])
        nc.gpsimd.dma_start(v_sbs[1][:, : nbmaxs[1] * D],
                            v_ap[1][:, : nbmaxs[1] * D])
        load_q(2)

        pending = []  # 2-deep SW pipeline keeps PE ahead of ACT

        def flush(pend):
            # PV matmuls for a finished group; on the chunk's last group also
            # emit the OT psum->sbuf cast-copy, the pacc fold (or direct P
            # DMA), and on the instance's last chunk the staged output DMAs.
            # Unmasked blocks go first so the PE never waits on the gpsimd
            # affine_select chain (all PV are full-width, so any block may
            # carry the start flag).
            order = ([u for u in range(pend["gn"]) if not pend["sel"][u]]
                     + [u for u in range(pend["gn"]) if pend["sel"][u]])
            for k, u in enumerate(order):
                sc = pend["blocks"][u]
                nc.tensor.matmul(
                    pend["ot_ps"][:, :TQ],
                    lhsT=pend["v_sb"][:, sc * D : (sc + 1) * D],
                    rhs=pend["p_sb"][:, u * TQ : (u + 1) * TQ],
                    start=(pend["first"] and k == 0),
                    stop=(pend["last"] and k == pend["gn"] - 1),
                )
            if not pend["first"]:
                # accumulate this group's P into the chunk's pacc6 (aliased
                # onto the first group's tile) -- emitted AFTER that tile's
                # PV reads so the WAR order stays correct
                nc.vector.tensor_tensor(
                    out=pend["pacc6"][:, : pend["gn"] * TQ],
                    in0=pend["pacc6"][:, : pend["gn"] * TQ],
                    in1=pend["p_sb"][:, : pend["gn"] * TQ],
                    op=mybir.AluOpType.add,
                )
            if pend["last"]:
                tcix, nb, ih = pend["tc"], pend["nb"], pend["ih"]
                osb, pacc_sb, pacc6 = pend["osb"], pend["pacc_sb"], pend["pacc6"]
                nc.vector.tensor_copy(
                    out=osb[:, tcix * TQ : (tcix + 1) * TQ],
                    in_=pend["ot_ps"][:, :TQ],
                )
                seg = pacc_sb[:, tcix * PW : (tcix + 1) * PW]
                if nb <= 2:
                    # masked P goes straight to DRAM; host sums nb*TQ cols
                    nc.sync.dma_start(
                        pacc_ap[ih][:, tcix * PW : tcix * PW + nb * TQ],
                        pacc6[:, : nb * TQ],
                    )
                else:
                    # fold the 4 lanes of pacc6 into 2 lanes (blocks beyond
                    # the first group were TT-accumulated into lanes mod 4;
                    # nb==3: the 4th lane was zeroed via memset)
                    nc.vector.tensor_tensor(
                        out=seg, in0=pacc6[:, :PW], in1=pacc6[:, PW : 2 * PW],
                        op=mybir.AluOpType.add,
                    )
                if ih == 2 * G - 1:
                    # last instance: per-chunk DMAs so the final (smallest)
                    # chunk is the only tail work
                    nc.sync.dma_start(
                        ot_ap[ih][:, tcix * TQ : (tcix + 1) * TQ],
                        osb[:, tcix * TQ : (tcix + 1) * TQ])
                    if nb > 2:
                        nc.sync.dma_start(
                            pacc_ap[ih][:, tcix * PW : (tcix + 1) * PW], seg)
                elif pend["last_of_inst"]:
                    i = ih // G
                    lo_t = t0s[i]
                    fold0 = next(t for t in range(NTQ)
                                 if nb_tabs[i][t] > 2)  # first folded chunk
                    nc.sync.dma_start(ot_ap[ih][:, lo_t * TQ :],
                                      osb[:, lo_t * TQ :])
                    nc.sync.dma_start(pacc_ap[ih][:, fold0 * PW :],
                                      pacc_sb[:, fold0 * PW :])

        def chunk_order(live, ih):
            # interleave big/small chunks to balance the PE-heavy (many
            # blocks) and ACT-heavy (call overhead) phases; instance 0 goes
            # ascending (small first chunk -> fast start after DMA); the
            # last instance ends on the smallest chunk to cut tail latency.
            if ih == 0:
                return list(live)
            out = []
            if ih == 2 * G - 1:
                rest = live[1:]
                i0, j0 = 0, len(rest) - 1
                while i0 <= j0:
                    out.append(rest[j0])
                    if i0 < j0:
                        out.append(rest[i0])
                    i0 += 1
                    j0 -= 1
                return out + [live[0]]
            i0, j0 = 0, len(live) - 1
            while i0 <= j0:
                out.append(live[i0])
                if i0 < j0:
                    out.append(live[j0])
                i0 += 1
                j0 -= 1
            return out

        for i in range(2):  # batch
            cb = c[i]
            nb_tab = nb_tabs[i]
            kT_sb = kT_sbs[i]
            v_sb = v_sbs[i]

            for j in range(G):
                ih = i * G + j
                if ih + 2 < 2 * G:
                    load_q(ih + 2)
                qT_sb = qT_sbs[ih]
                osb = osb_pool.tile([D, NTQ * TQ], f16)
                pacc_sb = pacc_pool.tile([TS, NTQ * PW], f16)

                live = [t for t in range(NTQ) if nb_tab[t] > 0]
                order = chunk_order(live, ih)
                for oi, tcix in enumerate(order):
                    nb = nb_tab[tcix]
                    is_last_chunk = oi == len(order) - 1
                    ot_ps = ot_psum.tile([D, TQ], f32)
                    blocks_all = list(range(nb - 1, -1, -1))  # descending s
                    pacc6 = None
                    g0 = 0
                    while g0 < nb:
                        gn = min(GRP, nb - g0)
                        blocks = blocks_all[g0 : g0 + gn]
                        st_ps = st_psum.tile([TS, GRP * TQ], f32)
                        offs = []
                        for u, sc in enumerate(blocks):
                            bv = TS * sc + cb - TQ * tcix
                            off = max(0, min(bv, TQ))
                            offs.append((sc, bv, off))
                            if off < TQ:
                                nc.tensor.matmul(
                                    st_ps[:, u * TQ + off : (u + 1) * TQ],
                                    lhsT=kT_sb[:, sc * TS : (sc + 1) * TS],
                                    rhs=qT_sb[:, tcix * TQ + off :
                                              (tcix + 1) * TQ],
                                    start=True,
                                    stop=True,
                                )
                        p_sb = p_pool.tile([TS, GRP * TQ], f16)
                        if g0 == 0:
                            pacc6 = p_sb
                            if nb == 3:
                                nc.gpsimd.memset(p_sb[:, 3 * TQ : 4 * TQ], 0.0)
                        estart = offs[0][2] if g0 == 0 else 0
                        nc.scalar.activation(
                            p_sb[:, estart : gn * TQ],
                            st_ps[:, estart : gn * TQ],
                            mybir.ActivationFunctionType.Exp,
                            scale=SCALE,
                        )
                        sel = [bv > -(TS - 1) for (sc, bv, off) in offs]
                        for u, (sc, bv, off) in enumerate(offs):
                            if bv > -(TS - 1):  # partially masked block
                                nc.gpsimd.affine_select(
                                    out=p_sb[:, u * TQ : (u + 1) * TQ],
                                    in_=p_sb[:, u * TQ : (u + 1) * TQ],
                                    pattern=[[1, TQ]],
                                    compare_op=mybir.AluOpType.is_ge,
                                    fill=0.0,
                                    base=-bv,
                                    channel_multiplier=-1,
                                )
                        if len(pending) >= 2:
                            flush(pending.pop(0))
                        pending.append({
                            "gn": gn, "nb": nb, "blocks": blocks, "sel": sel,
                            "p_sb": p_sb, "v_sb": v_sb, "ot_ps": ot_ps,
                            "pacc6": pacc6, "osb": osb, "pacc_sb": pacc_sb,
                            "ih": ih, "tc": tcix,
                            "first": g0 == 0, "last": g0 + gn >= nb,
                            "last_of_inst": (g0 + gn >= nb and is_last_chunk),
                        })
                        g0 += gn

        for pend in pending:
            flush(pend)

    nc.compile()
    return nc


def _get_program(c):
    key = tuple(int(x) for x in c)
    if key not in _compiled:
        _compiled[key] = _build_program(key)
    return _compiled[key]


def kernel(q, kv, key_padding_mask, _want_trace=False):
    import ml_dtypes

    bf16 = ml_dtypes.bfloat16
    q = np.asarray(q, dtype=np.float32)
    kv = np.asarray(kv, dtype=np.float32)
    mask = np.asarray(key_padding_mask).astype(bool)

    lengths = mask.sum(axis=1).astype(np.int64)
    # contiguous-prefix masks assumed (reference builds them that way)
    assert all(mask[b, : lengths[b]].all() and not mask[b, lengths[b]:].any()
               for b in range(B))
    c = tuple(int(SK - l) for l in lengths)
    nbmaxs = [(SK - 1 - ci) // TS + 1 for ci in c]
    NBK = max(nbmaxs)

    k_full = kv[:, :, 0]  # (B, SK, HK, D)
    v_full = kv[:, :, 1]

    k_bf = k_full.astype(bf16)
    v_16 = v_full.astype(np.float16)
    q_bf = q.astype(bf16)

    in_maps = []
    for core in range(N_CORES):
        hk = core
        qT = np.empty((2 * G, D, SQ), dtype=bf16)
        kT = np.zeros((2, D, NBK * TS), dtype=bf16)
        v_l = np.zeros((2, TS, NBK * D), dtype=np.float16)
        for i in range(B):
            nbk = nbmaxs[i]
            kT[i, :, : nbk * TS] = k_bf[i, : nbk * TS, hk, :].T
            # v chunked: v_l[i][p, sc*D + d] = v[i, sc*TS + p, hk, d]
            v_l[i, :, : nbk * D] = np.ascontiguousarray(
                v_16[i, : nbk * TS, hk, :].reshape(nbk, TS, D).transpose(1, 0, 2)
            ).reshape(TS, nbk * D)
            for j in range(G):
                qT[i * G + j] = q_bf[i, :, hk * G + j, :].T
        in_maps.append({
            "qT": np.ascontiguousarray(qT),
            "kT": kT,
            "v": v_l,
        })

    from concourse.bass_utils import run_bass_kernel_spmd

    nc = _get_program(c)
    res = run_bass_kernel_spmd(
        nc, in_maps, core_ids=list(range(N_CORES)),
        trace=_want_trace,
    )

    nb_tabs = [_nb_table(ci) for ci in c]
    out = np.empty((B, SQ, H, D), dtype=np.float32)
    for core in range(N_CORES):
        hk = core
        ot_core = res.results[core]["ot"]        # (2G, D, NTQ*TQ) f16
        pacc_core = res.results[core]["pacc"]    # (2G, TS, NTQ*PW) f16
        for i in range(B):
            nb_tab = nb_tabs[i]
            # den[t] from pacc segments: sum over partitions and live lanes
            pa = pacc_core[i * G : (i + 1) * G].astype(np.float32)
            pa = pa.reshape(G, TS, NTQ, 2, TQ)
            den = pa[:, :, :, 0].sum(axis=1)     # (G, NTQ, TQ)
            lane1 = pa[:, :, :, 1].sum(axis=1)
            for tc_i, nb_i in enumerate(nb_tab):
                if nb_i >= 2:
                    den[:, tc_i] += lane1[:, tc_i]
            den = den.reshape(G, SQ)
            ot = ot_core[i * G : (i + 1) * G].astype(np.float32)  # (G, D, SQ)
            with np.errstate(divide="ignore", invalid="ignore"):
                o = ot / den[:, None, :]          # (G, D, SQ)
            out[i, :, hk * G : (hk + 1) * G, :] = o.transpose(2, 0, 1)

    # rows that attend to nothing: reference softmax is uniform -> mean(V)
    for b in range(B):
        if c[b] > 0:
            vm = v_full[b].mean(axis=0)  # (HK, D)
            out[b, : c[b]] = np.repeat(vm, G, axis=0)[None]

    if _want_trace:
        return out, res
    return out
